# revision 13
# baseline (speedup 1.0000x reference)
"""Trainium2 Bass kernel for nn_Decoder (LSTM decoder with SE/HP MLP heads).

Strategy: pure data parallelism over batch (2048 -> 8 cores x 256).
Feature-major on-chip layout ([feature, batch]); weights stationary, batch
on the matmul moving dim. SE output projection folded into gate weights
(w2t = se_w2 @ w_ih.T, K=16), biases folded as fp8 contraction rows.

ALL matmuls are fp8 e4m3 DoubleRow (double-pumped, 0.5 cy/row): mixing DR
and regular matmuls costs ~400ns/mode-switch on the PE (measured), so even
tiny matmuls (u: K=2x2, p: K=8x2, x-part: K=9x2) run in DR form.

PSUM is one persistent [128, 4096] f32 tile; gate tile s lives at cols
[256s, 256s+256). Slot order (i0,i0',f0,f0',o0,o0',g0,g0', i1..g1') puts
all sigmoid tiles of one feature-half contiguous so one ACT instruction
covers 3 banks. Zeroing uses matmul start=True region semantics: ONE
start per bank per step (first-touch overwrites via the pending-zero
region mark; later matmuls accumulate). Bank 7 additionally time-muxes
the small matmuls (v, p, u) at partition-disjoint windows between the two
late g1 gate tiles; start=True marks there follow a strict order that the
emission order below preserves (see comments).

Per step (32 sequential steps):
  gates = DR(whh8, h8) + DR(w2tx9, T9)            [2048,256] (PSUM = 16x)
  i,f,o = sigmoid(g/16), g = tanh(g/16)           (ACT, 4 instrs, bf16 out)
  c = f*c + i*g; h8 = o*tanh(c)                   (DVE f32/bf16 -> fp8)
  v16 = DR(b18, h8); r9 = max(v16 + 16*c1hp, 0)   (fp8 DR + DVE)
  p256 = DR(hpw29, r9); s = p256 + 256*lp         (fp8 DR + DVE)
  lp = sigmoid(s/256 + hpb2) -> traj[t] (f32); lp8, lp256 via DVE
  u16 = max(DR(a1, lp8_ext), 0) -> T9             (fp8 DR + DVE)
"""

import json

import numpy as np
import ml_dtypes
from contextlib import ExitStack

import concourse.bass as bass
import concourse.mybir as mybir
import concourse.tile as tile
from concourse.bass import ts


def _fix_multiwait(bir_bytes: bytes) -> bytes:
    """Hoist excess sync waits onto injected EventSemaphore carriers
    (HW cap: 2 waits on EventSemaphore, 1 elsewhere; the Tile end-of-kernel
    drain can exceed this and the compiler rejects it)."""
    bir = json.loads(bir_bytes)
    for fn in bir.get("functions", []):
        for blk in fn.get("blocks", []):
            insts = blk.get("instructions")
            if not insts:
                continue
            out = []
            for inst in insts:
                si = inst.get("sync_info")
                waits = (si or {}).get("on_wait") or []
                cap = 2 if inst.get("opcode") == "EventSemaphore" else 1
                if len(waits) > cap:
                    excess, keep = waits[:-cap], waits[-cap:]
                    si["on_wait"] = keep
                    for i in range(0, len(excess), 2):
                        out.append({
                            "debug": inst.get("debug", 0),
                            "engine": inst["engine"],
                            "ins": [],
                            "name": f"{inst['name']}_xw{i}",
                            "opcode": "EventSemaphore",
                            "outs": [],
                            "sync_info": {"on_update": [], "on_wait": excess[i : i + 2]},
                        })
                out.append(inst)
            blk["instructions"] = out
    return json.dumps(bir).encode()

BF16 = ml_dtypes.bfloat16
F8 = ml_dtypes.float8_e4m3
F32 = np.float32

SEQ = 32
B = 2048
H = 512
E = 512
HID = 16
NCORES = 8
BL = B // NCORES  # 256 local batch
NG = 4 * H  # 2048 gate features
BN_EPS = 1e-5

_CACHE: dict = {}

# PSUM column offsets (f32 elements) inside the single [128, 4096] tile.
# Bank 7 (cols 3584:4096) also hosts the small matmuls:
W1 = 3584  # window 1 = gate slot 14's home
W2 = 3840  # window 2 = gate slot 15's home


def _build_nc(repeats: int = 1):
    nc = bass.Bass()
    dt = mybir.dt
    ACTF = mybir.ActivationFunctionType
    ALU = mybir.AluOpType
    DR = mybir.MatmulPerfMode.DoubleRow

    whh8_d = nc.dram_tensor("whh8", [2, 128, 2 * NG], dt.float8e4, kind="ExternalInput")
    w2tx10_d = nc.dram_tensor("w2tx10", [10, 2 * NG], dt.float8e4, kind="ExternalInput")
    b18a_d = nc.dram_tensor("b18a", [2, 128, 32], dt.float8e4, kind="ExternalInput")
    b18b_d = nc.dram_tensor("b18b", [2, 128, 32], dt.float8e4, kind="ExternalInput")
    a1a_d = nc.dram_tensor("a1a", [2, 32], dt.float8e4, kind="ExternalInput")
    a1b_d = nc.dram_tensor("a1b", [2, 32], dt.float8e4, kind="ExternalInput")
    hpw29_d = nc.dram_tensor("hpw29", [8, 32], dt.float8e4, kind="ExternalInput")
    c1hpA_d = nc.dram_tensor("c1hpA", [8, 1], dt.float32, kind="ExternalInput")
    c1hpB_d = nc.dram_tensor("c1hpB", [8, 1], dt.float32, kind="ExternalInput")
    hpb2_d = nc.dram_tensor("hpb2", [2, 1], dt.float32, kind="ExternalInput")
    lp2i_d = nc.dram_tensor("lp2i", [2, 2 * BL], dt.float8e4, kind="ExternalInput")
    t10c_d = nc.dram_tensor("t10c", [4, BL], dt.float8e4, kind="ExternalInput")
    lp256i_d = nc.dram_tensor("lp256i", [2, BL], dt.float32, kind="ExternalInput")
    h08_d = nc.dram_tensor("h08", [2, 128, 512], dt.float8e4, kind="ExternalInput")
    c0_d = nc.dram_tensor("c0", [2, 128, 512], dt.float32, kind="ExternalInput")
    traj_d = nc.dram_tensor("traj", [2, SEQ, BL], dt.float32, kind="ExternalOutput")

    with tile.TileContext(nc) as tc:
        with ExitStack() as ctx:
            singles = ctx.enter_context(tc.tile_pool(name="singles", bufs=1))
            gpool = ctx.enter_context(tc.tile_pool(name="gates", bufs=2))
            tpool = ctx.enter_context(tc.tile_pool(name="temps", bufs=3))
            psp = ctx.enter_context(tc.tile_pool(name="psp", bufs=1, space="PSUM"))

            PS = psp.tile([128, 4096], dt.float32, tag="PS", name="PS")

            # persistent weights
            whh8 = []
            for j in range(2):
                wt = singles.tile([128, 2 * NG], dt.float8e4, tag=f"whh8{j}", name=f"whh8{j}")
                nc.sync.dma_start(out=wt, in_=whh8_d[j, :, :])
                whh8.append(wt[:, :].rearrange("k (two m) -> k two m", two=2))
            w2t = singles.tile([10, 2 * NG], dt.float8e4, tag="w2t", name="w2t")
            nc.sync.dma_start(out=w2t, in_=w2tx10_d[:, :])
            w2tx10 = w2t[:, :].rearrange("k (two m) -> k two m", two=2)
            b18 = []
            for nm, dd in (("b18a", b18a_d), ("b18b", b18b_d)):
                pair = []
                for j in range(2):
                    bt = singles.tile([128, 32], dt.float8e4, tag=f"{nm}{j}", name=f"{nm}{j}")
                    nc.sync.dma_start(out=bt, in_=dd[j, :, :])
                    pair.append(bt[:, :].rearrange("k (two m) -> k two m", two=2))
                b18.append(pair)
            a1a_t = singles.tile([2, 32], dt.float8e4, tag="a1a", name="a1a")
            nc.sync.dma_start(out=a1a_t, in_=a1a_d[:, :])
            a1a = a1a_t[:, :].rearrange("k (two m) -> k two m", two=2)
            a1b_t = singles.tile([2, 32], dt.float8e4, tag="a1b", name="a1b")
            nc.sync.dma_start(out=a1b_t, in_=a1b_d[:, :])
            a1b = a1b_t[:, :].rearrange("k (two m) -> k two m", two=2)
            hpw_t = singles.tile([8, 32], dt.float8e4, tag="hpw", name="hpw")
            nc.sync.dma_start(out=hpw_t, in_=hpw29_d[:, :])
            hpw29 = hpw_t[:, :].rearrange("k (two m) -> k two m", two=2)
            c1hpA = singles.tile([8, 1], dt.float32, tag="c1hpA", name="c1hpA")
            nc.sync.dma_start(out=c1hpA, in_=c1hpA_d[:, :])
            c1hpB = singles.tile([8, 1], dt.float32, tag="c1hpB", name="c1hpB")
            nc.sync.dma_start(out=c1hpB, in_=c1hpB_d[:, :])
            hpb2 = singles.tile([2, 1], dt.float32, tag="hpb2", name="hpb2")
            nc.sync.dma_start(out=hpb2, in_=hpb2_d[:, :])

            # persistent state
            lp2 = singles.tile([2, 2 * BL], dt.float8e4, tag="lp2", name="lp2")
            nc.sync.dma_start(out=lp2, in_=lp2i_d[:, :])
            lp2v = lp2[:, :].rearrange("k (two n) -> k two n", two=2)
            T10 = singles.tile([10, 2 * BL], dt.float8e4, tag="T10", name="T10")
            nc.sync.dma_start(out=T10[6:10, BL : 2 * BL], in_=t10c_d[:, :])
            t10v = T10[:, :].rearrange("k (two n) -> k two n", two=2)
            lp256 = singles.tile([2, BL], dt.float32, tag="lp256", name="lp256")
            nc.sync.dma_start(out=lp256, in_=lp256i_d[:, :])
            traj = singles.tile([2, SEQ * BL], dt.float32, tag="traj", name="traj")
            hb2, cst = {0: [], 1: []}, []
            for j in range(2):
                t_b = singles.tile([128, 512], dt.float8e4, tag=f"hbA{j}", name=f"hbA{j}")
                nc.sync.dma_start(out=t_b, in_=h08_d[j, :, :])
                hb2[0].append(t_b)
                t_b2 = singles.tile([128, 512], dt.float8e4, tag=f"hbB{j}", name=f"hbB{j}")
                hb2[1].append(t_b2)
                t_c = singles.tile([128, 512], dt.float32, tag=f"c{j}", name=f"c{j}")
                nc.sync.dma_start(out=t_c, in_=c0_d[j, :, :])
                cst.append(t_c)

            def hview(t, jj):
                return hb2[t % 2][jj][:, :].rearrange("k (two b) -> k two b", two=2)

            def mm(out, lhsT, rhs, start):
                nc.tensor.matmul(out, lhsT, rhs, start=start, stop=True,
                                 perf_mode=DR, skip_group_check=True)

            def dr1(t, s):
                # first touch of each bank this step: start=True (even slots)
                mm(PS[:, ts(s, 256)], whh8[0][:, :, ts(s, 128)], hview(t, 0),
                   start=(s % 2 == 0))

            def dr2(t, s):
                mm(PS[:, ts(s, 256)], whh8[1][:, :, ts(s, 128)], hview(t, 1),
                   start=False)

            def xclose(s):
                mm(PS[:, ts(s, 256)], w2tx10[:, :, ts(s, 128)], t10v, start=False)

            def emit_u(t):
                # uA carries a fresh start mark for bank 7 (v/p data read by
                # now via the lp8 chain); uB's bytes overwrite via the mark.
                mm(PS[0:16, W2 : W2 + BL], a1a, lp2v, start=True)
                mm(PS[0:16, W1 : W1 + BL], a1b, lp2v, start=False)
                nc.vector.tensor_scalar(
                    T10[0:10, 0:BL], PS[0:10, W2 : W2 + BL], 0.0, None, op0=ALU.max
                )
                nc.vector.tensor_scalar(
                    T10[0:6, BL : 2 * BL], PS[0:6, W1 : W1 + BL], 0.0, None, op0=ALU.max
                )

            def emit_tail_gates(t):
                # late g1 tiles (slots 14, 15); T14's start re-marks bank 7
                # after relu_u read u (emission order guarantees this).
                mm(PS[:, ts(14, 256)], whh8[0][:, :, ts(14, 128)], hview(t, 0),
                   start=True)
                mm(PS[:, ts(14, 256)], whh8[1][:, :, ts(14, 128)], hview(t, 1),
                   start=False)
                xclose(14)
                mm(PS[:, ts(15, 256)], whh8[0][:, :, ts(15, 128)], hview(t, 0),
                   start=False)
                mm(PS[:, ts(15, 256)], whh8[1][:, :, ts(15, 128)], hview(t, 1),
                   start=False)
                xclose(15)

            def elem_pre(t, j, gsS, gsG):
                i_t, f_t, g_t = gsS[:, 0:512], gsS[:, 512:1024], gsG[:, 0:512]
                t_ig = tpool.tile([128, 512], dt.bfloat16, tag="tig", name=f"tig{t}_{j}")
                t_fc = tpool.tile([128, 512], dt.float32, tag="tfc", name=f"tfc{t}_{j}")
                nc.vector.tensor_mul(t_ig, i_t, g_t)
                nc.vector.tensor_mul(t_fc, f_t, cst[j])
                nc.vector.tensor_add(cst[j], t_fc, t_ig)

            def elem_post(t, j, gsS):
                o_t = gsS[:, 1024:1536]
                t_tc = tpool.tile([128, 512], dt.bfloat16, tag="ttc", name=f"ttc{t}_{j}")
                nc.scalar.activation(t_tc, cst[j], ACTF.Tanh)
                nc.vector.tensor_mul(hb2[(t + 1) % 2][j], o_t, t_tc)

            def body():
                # prologue: u(0) + full gate prefill for t=0 from h0
                emit_u(0)
                for s in range(14):
                    dr1(0, s)
                for s in range(14):
                    dr2(0, s)
                emit_tail_gates(0)

                for t in range(SEQ):
                    hasnxt = t + 1 < SEQ
                    for s in range(14):
                        xclose(s)
                    gsA = gpool.tile([128, 1536], dt.bfloat16, tag="gsA", name=f"gsA{t}")
                    nc.scalar.activation(gsA, PS[:, 0:1536], ACTF.Sigmoid, scale=1.0 / 16.0)
                    gsG0 = gpool.tile([128, 512], dt.bfloat16, tag="gsG0", name=f"gsG0{t}")
                    nc.scalar.activation(gsG0, PS[:, 1536:2048], ACTF.Tanh, scale=1.0 / 16.0)
                    gsB = gpool.tile([128, 1536], dt.bfloat16, tag="gsB", name=f"gsB{t}")
                    nc.scalar.activation(gsB, PS[:, 2048:3584], ACTF.Sigmoid, scale=1.0 / 16.0)
                    gsG1 = gpool.tile([128, 512], dt.bfloat16, tag="gsG1", name=f"gsG1{t}")
                    nc.scalar.activation(gsG1, PS[:, 3584:4096], ACTF.Tanh, scale=1.0 / 16.0)

                    elem_pre(t, 0, gsA, gsG0)
                    elem_post(t, 0, gsA)
                    elem_pre(t, 1, gsB, gsG1)
                    elem_post(t, 1, gsB)

                    # ---- tail: prefill t+1 + HP head chain ----
                    if hasnxt:
                        for s in range(14):
                            dr1(t + 1, s)
                    # vA starts bank 7's cycle (G1(t) has read tiles 14/15)
                    mm(PS[0:16, W2 : W2 + BL], b18[0][0], hview(t + 1, 0), start=True)
                    if hasnxt:
                        for s in range(14):
                            dr2(t + 1, s)
                    mm(PS[0:16, W2 : W2 + BL], b18[0][1], hview(t + 1, 1), start=False)
                    mm(PS[0:16, W1 : W1 + BL], b18[1][0], hview(t + 1, 0), start=False)
                    mm(PS[0:16, W1 : W1 + BL], b18[1][1], hview(t + 1, 1), start=False)
                    r9 = tpool.tile([8, 2 * BL], dt.float8e4, tag="r9", name=f"r9{t}")
                    nc.vector.tensor_scalar(
                        r9[0:8, 0:BL], PS[0:8, W2 : W2 + BL], c1hpA[:, :], 0.0,
                        op0=ALU.add, op1=ALU.max,
                    )
                    nc.vector.tensor_scalar(
                        r9[0:8, BL : 2 * BL], PS[0:8, W1 : W1 + BL], c1hpB[:, :], 0.0,
                        op0=ALU.add, op1=ALU.max,
                    )
                    r9v = r9[:, :].rearrange("k (two n) -> k two n", two=2)
                    # p re-marks bank 7 (v read via r9 RAW dep)
                    mm(PS[0:16, W1 : W1 + BL], hpw29, r9v, start=True)
                    s_t = tpool.tile([2, BL], dt.float32, tag="st", name=f"st{t}")
                    nc.vector.tensor_add(s_t, PS[0:2, W1 : W1 + BL], lp256)
                    nc.scalar.activation(
                        traj[:2, ts(t, BL)], s_t, ACTF.Sigmoid,
                        bias=hpb2, scale=1.0 / 256.0,
                    )
                    nc.vector.tensor_scalar_mul(lp256, traj[:2, ts(t, BL)], 256.0)
                    nc.vector.tensor_copy(lp2[0:2, 0:BL], traj[:2, ts(t, BL)])
                    if hasnxt:
                        emit_u(t + 1)
                        emit_tail_gates(t + 1)

            if repeats == 1:
                body()
            else:
                with tc.For_i(0, repeats, 1):
                    body()

            nc.sync.dma_start(
                out=traj_d[:, :, :].rearrange("p t b -> p (t b)"), in_=traj[:2, :]
            )
    patched = _fix_multiwait(nc.to_json_bytes())
    nc.to_json_bytes = lambda: patched
    return nc


def _pack_half(x_t):
    # [512, BL] feature-major -> [2, 128, 2*BL]: tile j holds feature-tiles
    # 2j (cols 0:BL) and 2j+1 (cols BL:2BL)
    xr = x_t.reshape(4, 128, BL)
    return np.stack(
        [np.concatenate([xr[2 * j], xr[2 * j + 1]], axis=1) for j in range(2)]
    )


# PSUM slot (s=0..15) -> original 128-row gate block. Reference gate order is
# (i, f, g, o): i=blocks 0-3, f=4-7, g=8-11, o=12-15. Slot order is
# [i0,i0',f0,f0',o0,o0',g0,g0', i1,i1',f1,f1',o1,o1',g1,g1'].
_SLOT_BLOCKS = [0, 1, 4, 5, 12, 13, 8, 9, 2, 3, 6, 7, 14, 15, 10, 11]
_PERM = np.concatenate([np.arange(128) + 128 * b for b in _SLOT_BLOCKS])


def _q8(x):
    return np.asarray(x, dtype=np.float64).astype(F8)


def _dr_pack(rows):
    """[2G, M] -> [G, 2*M]: out[k, i*M+m] = rows[G*i + k, m]."""
    rows = np.asarray(rows)
    g = rows.shape[0] // 2
    return np.concatenate([rows[0:g], rows[g : 2 * g]], axis=1)


def _host_prep(inputs):
    f = lambda k: np.asarray(inputs[k], dtype=np.float64)
    se_w1, se_b1 = f("se_w1"), f("se_b1")
    se_g, se_bt, se_m, se_v = f("se_g"), f("se_bt"), f("se_m"), f("se_v")
    se_w2, se_b2 = f("se_w2"), f("se_b2")
    w_ih, w_hh, b_ih, b_hh = f("w_ih"), f("w_hh"), f("b_ih"), f("b_hh")
    hp_w1, hp_b1 = f("hp_w1"), f("hp_b1")
    hp_g, hp_bt, hp_m, hp_v = f("hp_g"), f("hp_bt"), f("hp_m"), f("hp_v")
    hp_w2, hp_b2 = f("hp_w2"), f("hp_b2")

    s_se = se_g / np.sqrt(se_v + BN_EPS)
    a1 = se_w1 * s_se[None, :]  # [2, 16]
    c1_se = (se_b1 - se_m) * s_se + se_bt  # [16]
    s_hp = hp_g / np.sqrt(hp_v + BN_EPS)
    b1 = hp_w1 * s_hp[None, :]  # [512, 16]
    c1_hp = (hp_b1 - hp_m) * s_hp + hp_bt  # [16]

    w2t = (se_w2 @ w_ih.T)[:, _PERM]  # [16, 2048]
    b_eff = (b_ih + b_hh + w_ih @ se_b2)[_PERM]  # [2048]
    w_hh_p = w_hh[_PERM, :]  # [2048, 512]

    # whh8 [2, 128, 2, NG]: [j, k, i, m] = e4m3(16*w_hh_p[m, (2j+i)*128+k])
    wT = (16.0 * w_hh_p.T).reshape(4, 128, NG)  # [kk, k, m]
    whh8 = np.stack(
        [np.stack([wT[2 * j + i] for i in range(2)], axis=1) for j in range(2)]
    )

    # x-part [18 rows]: w2t (UNSCALED: u comes in as 16*u) + bias rows
    # matched to T9 consts (16.0, 1.0).
    b_hi = _q8(b_eff).astype(np.float64)
    b_lo = 16.0 * (b_eff - b_hi)
    zr = np.zeros((2, NG))
    w2tx = np.concatenate([w2t, b_hi[None, :], b_lo[None, :], zr], axis=0)  # [20, NG]
    w2tx10 = _q8(_dr_pack(w2tx))  # [10, 2*NG]

    # v weights (x16), split into hp-col halves for DR + partition-0 outputs
    bT = (16.0 * b1).reshape(4, 128, HID)  # [kk, k, c]
    b18 = np.stack(
        [np.stack([bT[2 * j + i] for i in range(2)], axis=1) for j in range(2)]
    )  # [2, 128, 2, 16]
    def padM(x, M=16):
        pad = np.zeros(x.shape[:-1] + (M - x.shape[-1],))
        return np.concatenate([x, pad], axis=-1)
    b18a = _q8(padM(b18[..., 0:8]).reshape(2, 128, 32))
    b18b = _q8(padM(b18[..., 8:16]).reshape(2, 128, 32))

    # u weights: rows = [16*A1_x, 16*A1_y, c_hi, c_lo] against lp2 rows
    # (lp_x, lp_y, 1, 1); split into u-col halves 0:9 / 9:16.
    c_hi = _q8(16.0 * c1_se).astype(np.float64)
    c_lo = 16.0 * c1_se - c_hi
    a1ext = np.stack([16.0 * a1[0], 16.0 * a1[1], c_hi, c_lo])  # [4, 16]
    a1a = _q8(_dr_pack(padM(a1ext[:, 0:10])))  # [2, 32]
    a1b = _q8(_dr_pack(padM(a1ext[:, 10:16])))  # [2, 32]

    hpw29 = _q8(_dr_pack(padM(16.0 * hp_w2)))  # [8, 32]

    rep = {
        "whh8": np.ascontiguousarray(_q8(whh8)).reshape(2, 128, 2 * NG),
        "w2tx10": np.ascontiguousarray(w2tx10),
        "b18a": np.ascontiguousarray(b18a),
        "b18b": np.ascontiguousarray(b18b),
        "a1a": np.ascontiguousarray(a1a),
        "a1b": np.ascontiguousarray(a1b),
        "hpw29": np.ascontiguousarray(hpw29),
        "c1hpA": (16.0 * c1_hp[0:8]).astype(F32).reshape(8, 1),
        "c1hpB": (16.0 * c1_hp[8:16]).astype(F32).reshape(8, 1),
        "hpb2": hp_b2.astype(F32).reshape(2, 1),
    }
    t10c = np.zeros((4, BL), dtype=F8)
    t10c[0] = F8(16.0)
    t10c[1] = F8(1.0)
    rep["t10c"] = t10c

    last_pos = np.asarray(inputs["last_pos"], dtype=np.float64)
    h0 = np.asarray(inputs["hh"], dtype=np.float64)[0]
    c0 = np.asarray(inputs["ch"], dtype=np.float64)[0]
    in_maps = []
    for c in range(NCORES):
        rows = slice(c * BL, (c + 1) * BL)
        h0t = np.ascontiguousarray(h0[rows].T)  # [512, BL]
        c0t = np.ascontiguousarray(c0[rows].T)
        m = dict(rep)
        lp0t = np.ascontiguousarray(last_pos[rows].T)  # [2, BL]
        lp2i = np.empty((2, 2 * BL), dtype=F8)
        lp2i[:, 0:BL] = _q8(lp0t)
        lp2i[:, BL:] = F8(1.0)
        m["lp2i"] = lp2i
        m["lp256i"] = (256.0 * lp0t).astype(F32)
        m["h08"] = _q8(_pack_half(h0t))
        m["c0"] = _pack_half(c0t).astype(F32)
        in_maps.append(m)
    return in_maps


def _get_runner(repeats: int = 1):
    """Build (once) a persistent jitted SPMD runner over 8 cores."""
    key = ("runner", repeats)
    if key in _CACHE:
        return _CACHE[key]

    import jax
    from jax.sharding import Mesh, PartitionSpec, NamedSharding
    from jax.experimental.shard_map import shard_map
    from concourse import bass2jax, mybir as _mb

    nc = _build_nc(repeats)
    bass2jax.install_neuronx_cc_hook()

    partition_name = nc.partition_id_tensor.name if nc.partition_id_tensor else None
    in_names, out_names, out_avals, zero_shapes = [], [], [], []
    for alloc in nc.m.functions[0].allocations:
        if not isinstance(alloc, _mb.MemoryLocationSet):
            continue
        name = alloc.memorylocations[0].name
        if alloc.kind == "ExternalInput":
            if name != partition_name:
                in_names.append(name)
        elif alloc.kind == "ExternalOutput":
            out_names.append(name)
            shape = tuple(alloc.tensor_shape)
            dtype = _mb.dt.np(alloc.dtype)
            out_avals.append(jax.core.ShapedArray(shape, dtype))
            zero_shapes.append((shape, dtype))
    n_params = len(in_names)
    all_names = in_names + out_names
    if partition_name is not None:
        all_names = all_names + [partition_name]
    donate = tuple(range(n_params, n_params + len(out_names)))

    def _body(*args):
        operands = list(args)
        if partition_name is not None:
            operands.append(bass2jax.partition_id_tensor())
        outs = bass2jax._bass_exec_p.bind(
            *operands,
            out_avals=tuple(out_avals),
            in_names=tuple(all_names),
            out_names=tuple(out_names),
            lowering_input_output_aliases=(),
            sim_require_finite=True,
            sim_require_nnan=True,
            nc=nc,
        )
        return tuple(outs)

    devices = jax.devices()[:NCORES]
    mesh = Mesh(np.asarray(devices), ("core",))
    spec = PartitionSpec("core")
    sharded = jax.jit(
        shard_map(
            _body,
            mesh=mesh,
            in_specs=(spec,) * (n_params + len(out_names)),
            out_specs=(spec,) * len(out_names),
            check_rep=False,
        ),
        donate_argnums=donate,
        keep_unused=True,
    )
    sharding = NamedSharding(mesh, spec)

    def stage(in_maps):
        concat = [
            np.concatenate([np.asarray(m[name]) for m in in_maps], axis=0)
            for name in in_names
        ]
        return [jax.device_put(a, sharding) for a in concat]

    def exec_(staged):
        zeros = [
            jax.device_put(np.zeros((NCORES * s[0], *s[1:]), d), sharding)
            for s, d in zero_shapes
        ]
        outs = sharded(*staged, *zeros)
        outs = [np.asarray(o) for o in outs]
        return {
            name: outs[i].reshape(NCORES, *out_avals[i].shape)
            for i, name in enumerate(out_names)
        }

    _CACHE[key] = (stage, exec_)
    return _CACHE[key]


def kernel(**inputs) -> np.ndarray:
    stage, exec_ = _get_runner()
    staged = stage(_host_prep(inputs))
    per_core = exec_(staged)["traj"]  # [8, 2, 32, BL]
    out = per_core.transpose(2, 0, 3, 1).reshape(SEQ, B, 2)
    return np.ascontiguousarray(out.astype(np.float32))
